# revision 71
# baseline (speedup 1.0000x reference)
"""Causal self-attention on 8 TRN2 NeuronCores.

Sharding: core c handles batch b=c//2, head-group g=c%2 (heads g*8..g*8+7).
Each core computes the qkv projection for its 8 heads, causal attention, and
a partial out-projection (its heads' columns of w_out). Host sums the two
partial outputs per batch. All layout transposes are done host-side.

On-chip (per core), P=128 partitions, bf16 matmul operands, f32 PSUM:
  xT    [1024(c), 2048(t)]   x[b] transposed
  wqkvT [1024(c), 1536(f)]   f = [qT 512 | kT 512 | vT 512] for this group
  woutT [512(dv), 1024(o)]   w_out columns for this group, transposed
  scoresT[j, i] = sum_d kT[d,j] qT[d,i]  (softmax runs over partition dim j)

QK matmuls have contraction K=64 (head dim), which uses only half the
128x128 PE array; matmuls on different 64-row PE tiles execute
concurrently (measured ~2x). Heads are processed in pairs (even head ->
row tile (0,0), odd -> (64,0)), and each j-tile's scores for BOTH heads
go into ONE [128, A|B] psum tile finished by ONE exp — so both heads'
next QK matmuls become ready simultaneously and the readiness-driven
scheduler keeps the T0/T8 pair adjacent (that adjacency is what makes
them overlap). PV is K=128 (full array) and already time-optimal: matmul
time scales with the output free size N only, so splitting K across row
tiles would gain nothing. The softmax denominator comes from a ones
column appended to v (psum row 64); per pair, denominators are inverted
with reciprocal_approx_fast (DVE) and broadcast across partitions via a
DRAM bounce, then multiplied into oT. fp8 was evaluated and rejected:
attention output is a weighted mean (std ~0.1), so e4m3's ~3% per-element
quantization of p/v does NOT average down relative to the signal and
blows the 2e-2 gate.

Emission order is the schedule: QKV for later blocks and earlier
out-projection tiles are paced into the attention stream as PE filler
(the ACT-bound exp chain throttles attention via the 2-deep qk psum
ring), DMAs are ordered by first need on the single ~300GB/s hardware
queue (~8.7us fixed startup), and out-projection units are reserved for
ib3's pair boundaries to cover the tail normalize chains.
"""

import math
import numpy as np
import ml_dtypes

B, T, D, H, HD = 4, 2048, 1024, 16, 64
P = 128
HPG = 8          # heads per group
FG = HPG * HD    # 512 features per group
NCC = D // P     # 8 contraction chunks
NTB = 4          # t-blocks of 512
NTT = 16         # t-tiles of 128
NIB = 4          # i-blocks of 512
SCALE = 1.0 / math.sqrt(HD)
NEG = -1.0e30

_CACHE = {}


def _import_concourse():
    """Make concourse importable in environments where it isn't on sys.path."""
    try:
        import concourse.bass  # noqa: F401
        return
    except ImportError:
        pass
    import sys, os
    for p in ("/opt/trn_rl_repo", "/root/.axon_site/_ro/trn_rl_repo"):
        if os.path.isdir(p) and p not in sys.path:
            sys.path.insert(0, p)
    import concourse.bass  # noqa: F401


def _build_nc():
    _import_concourse()
    from concourse import bacc
    import concourse.mybir as mybir
    import concourse.tile as tile
    from contextlib import ExitStack

    BF = mybir.dt.bfloat16
    F32 = mybir.dt.float32

    nc = bacc.Bacc("TRN2", target_bir_lowering=False, debug=False, num_devices=8)
    xT = nc.dram_tensor("xT", [D, T], BF, kind="ExternalInput").ap()
    wqkvT = nc.dram_tensor("wqkvT", [D, 3 * FG], BF, kind="ExternalInput").ap()
    woutT = nc.dram_tensor("woutT", [FG, D], BF, kind="ExternalInput").ap()
    maskd = nc.dram_tensor("maskd", [P, P], F32, kind="ExternalInput").ap()
    out = nc.dram_tensor("out", [T, D], F32, kind="ExternalOutput").ap()

    with tile.TileContext(nc) as tc, ExitStack() as ctx:
        singles = ctx.enter_context(tc.tile_pool(name="singles", bufs=1))
        ptp = ctx.enter_context(tc.tile_pool(name="pt", bufs=8))
        ssp = ctx.enter_context(tc.tile_pool(name="ss", bufs=2))
        nmp = ctx.enter_context(tc.tile_pool(name="nm", bufs=10))
        bcp = ctx.enter_context(tc.tile_pool(name="bc", bufs=4))
        drp = ctx.enter_context(tc.tile_pool(name="dr", bufs=4, space="DRAM"))
        yp = ctx.enter_context(tc.tile_pool(name="y", bufs=3))
        ps_mm = ctx.enter_context(tc.tile_pool(name="ps_mm", bufs=2, space="PSUM"))
        ps_qk = ctx.enter_context(tc.tile_pool(name="ps_qk", bufs=2, space="PSUM"))
        ps_pv = ctx.enter_context(tc.tile_pool(name="ps_pv", bufs=1, space="PSUM"))

        wq_sb = singles.tile([P, NCC, 3 * FG], BF)
        wq_src = wqkvT.rearrange("(cc p) f -> p cc f", p=P)
        mask_sb = singles.tile([P, P], F32)
        wo_sb = singles.tile([P, 4, D], BF)
        xt_full = singles.tile([P, NCC, T], BF)  # all of x, loaded once

        qk_sb = singles.tile([P, 8, T], BF)              # f-tiles 0..3 = q, 4..7 = k
        vp_sb = singles.tile([P, NTT, HPG, HD + 1], BF)  # [v_h | ones]
        oT_sb = singles.tile([P, 4, T], BF)              # attn out, [dv, t]
        nc.vector.memset(vp_sb[:, :, :, HD:HD + 1], 1.0)

        import concourse.bass as _b

        # ---- emission helpers ----
        def emit_qkv_block(tb):
            """Return one thunk per psum group (8 q/k + 4 v) for x-block tb."""
            thunks = []
            for ft in range(8):  # q then k feature tiles, output [f=128, t=512]
                def qk_group(ft=ft, tb=tb):
                    ps = ps_mm.tile([P, 512], F32)
                    for cc in range(NCC):
                        nc.tensor.matmul(
                            ps,
                            lhsT=wq_sb[:, cc, ft * P:(ft + 1) * P],
                            rhs=xt_full[:, cc, tb * 512:(tb + 1) * 512],
                            start=(cc == 0),
                            stop=(cc == NCC - 1),
                        )
                    nc.vector.tensor_copy(
                        out=qk_sb[:, ft, tb * 512:(tb + 1) * 512], in_=ps
                    )
                thunks.append(qk_group)
            for tl in range(4):  # v in [t, dv] orientation, output [t=128, dv=512]
                def v_group(tl=tl, tb=tb):
                    tt = tb * 4 + tl
                    ps = ps_mm.tile([P, FG], F32)
                    for cc in range(NCC):
                        nc.tensor.matmul(
                            ps,
                            lhsT=xt_full[:, cc, tt * P:(tt + 1) * P],
                            rhs=wq_sb[:, cc, 2 * FG:3 * FG],
                            start=(cc == 0),
                            stop=(cc == NCC - 1),
                        )
                    nc.vector.tensor_copy(
                        out=vp_sb[:, tt, :, 0:HD],
                        in_=ps.rearrange("p (h d) -> p h d", h=HPG),
                    )
                thunks.append(v_group)
            return thunks

        def emit_outproj_tt(tt):
            yt = yp.tile([P, 1024], F32)
            for ob in range(2):
                ps = ps_mm.tile([P, 512], F32)
                for dc in range(4):
                    nc.tensor.matmul(
                        ps,
                        lhsT=oT_sb[:, dc, tt * P:(tt + 1) * P],
                        rhs=wo_sb[:, dc, ob * 512:(ob + 1) * 512],
                        start=(dc == 0),
                        stop=(dc == 3),
                    )
                nc.vector.tensor_copy(yt[:, ob * 512:(ob + 1) * 512], ps)
            nc.sync.dma_start(out=out[tt * P:(tt + 1) * P, :], in_=yt)

        def emit_qk_jt(ib, hp, jt):
            """One j-tile of scores for BOTH heads of pair hp in one psum
            tile [A cols 0:512 | B cols 512:1024], finished by ONE exp so
            both heads' downstream work becomes ready simultaneously (the
            scheduler then runs the T0/T8 matmul pair adjacently, which is
            what makes them execute concurrently on the PE row tiles)."""
            r = jt - 4 * ib
            c0 = P * r if r > 0 else 0
            fq, fk = hp, 4 + hp
            qk = ps_qk.tile([P, 1024], F32)
            i0 = ib * 512
            for (po, off) in ((0, 0), (64, 512)):  # head A -> T0, head B -> T8
                nc.tensor.matmul(
                    qk[:, off + c0:off + 512],
                    lhsT=qk_sb[po:po + 64, fk, jt * P:(jt + 1) * P],
                    rhs=qk_sb[po:po + 64, fq, i0 + c0:i0 + 512],
                    start=True, stop=True,
                )
            if r >= 0:  # mask the diagonal 128x128 sub-blocks
                nc.vector.tensor_add(
                    qk[:, c0:c0 + P], qk[:, c0:c0 + P], mask_sb)
                nc.vector.tensor_add(
                    qk[:, 512 + c0:512 + c0 + P],
                    qk[:, 512 + c0:512 + c0 + P], mask_sb)
            pt = ptp.tile([P, 1024], BF)
            nc.scalar.activation(
                out=pt[:, c0:1024], in_=qk[:, c0:1024],
                func=mybir.ActivationFunctionType.Exp, scale=SCALE,
            )
            return pt, c0

        def emit_pv_jt(ib, h, jt, pt, off, PV, njt):
            """PV for head h, j-tile jt: full-K (128) accumulation into this
            head's psum bank. Time-optimal already (matmul time ~ N)."""
            r = jt - 4 * ib
            c0 = P * r if r > 0 else 0
            nc.tensor.matmul(
                PV[:, c0:512],
                lhsT=vp_sb[:, jt, h, :],
                rhs=pt[:, off + c0:off + 512],
                start=(jt == 0), stop=(jt == njt - 1),
            )

        def emit_combine(ib, h, PV, nms):
            """Evict PV psum: nm in [65,512] — numerators in rows 0..63, the
            softmax denominator row at row 64."""
            nm = nmp.tile([HD + 1, 512], F32)
            nms[h] = nm
            nc.vector.tensor_copy(nm, PV)

        def normalize_pair(ib, hA, hB, nms):
            """Per-pair normalize right after the combines, fully on-chip
            with no DMA hops: a K=1 outer-product matmul broadcasts the
            denominator row across 64 psum partitions, approx reciprocal
            (DVE, psum in), then the normalize muls into oT."""
            isl = slice(ib * 512, (ib + 1) * 512)
            stc = ssp.tile([2, 512], F32)
            st2 = ssp.tile([2, 512], F32)
            sd = drp.tile([2, 512], F32)
            for q, h in enumerate((hA, hB)):
                nc.sync.dma_start(
                    out=stc[q:q + 1, :], in_=nms[h][HD:HD + 1, :])
            nc.vector.reciprocal_approx_fast(out=st2, in_=stc)
            nc.sync.dma_start(out=sd, in_=st2)
            bc = bcp.tile([64, 2, 512], F32)
            bcast = _b.AP(
                tensor=sd.tensor, offset=sd.offset,
                ap=[[0, 64], [512, 2], list(sd.ap[-1])],
            )
            nc.sync.dma_start(out=bc, in_=bcast)
            for q, h in enumerate((hA, hB)):
                po = (h % 2) * 64
                nc.vector.tensor_mul(
                    oT_sb[po:po + 64, h // 2, isl], nms[h][0:HD, :],
                    bc[:, q, :])

        # ---- top-level emission ----
        # DMA priority order: x block 0, q+k weight sections (unblocks the
        # first projection groups and attention), then the rest
        # single hw DMA queue at ~300GB/s, FIFO: order by first need
        xt_src = xT.rearrange("(cc p) t -> p cc t", p=P)
        nc.sync.dma_start(out=xt_full[:, :, 0:512], in_=xt_src[:, :, 0:512])
        nc.sync.dma_start(out=wq_sb[:, :, 0:FG], in_=wq_src[:, :, 0:FG])
        nc.sync.dma_start(
            out=wq_sb[:, :, FG:2 * FG], in_=wq_src[:, :, FG:2 * FG])
        nc.sync.dma_start(out=mask_sb, in_=maskd)
        nc.sync.dma_start(
            out=wq_sb[:, :, 2 * FG:3 * FG], in_=wq_src[:, :, 2 * FG:3 * FG])
        for tb in range(1, 4):
            nc.sync.dma_start(
                out=xt_full[:, :, tb * 512:(tb + 1) * 512],
                in_=xt_src[:, :, tb * 512:(tb + 1) * 512])
        blk0 = emit_qkv_block(0)
        for i in (0, 4, 8, 9):  # q0, k0, v0, v1 (pair 0 of ib 0)
            blk0[i]()

        # filler units per ib (beyond the explicit ib0 qkv scheduling)
        def op_unit(tt):
            return lambda: emit_outproj_tt(tt)

        for ib in range(NIB):
            njt = 4 * ib + 4
            if ib == 0:
                filler = list(emit_qkv_block(1))
            elif ib == 1:
                nc.sync.dma_start(
                    out=wo_sb, in_=woutT.rearrange("(dc p) o -> p dc o", p=P))
                filler = list(emit_qkv_block(2))
            elif ib == 2:
                filler = list(emit_qkv_block(3))
            else:
                filler = [op_unit(t) for t in range(8)]
            slots = 4 * (njt + 1)
            nms = {}
            done = 0
            slot = 0

            def pace(filler=filler, slots=slots):
                nonlocal done, slot
                slot += 1
                want = min(len(filler), (slot * len(filler) + slots - 1) // slots)
                while done < want:
                    filler[done]()
                    done += 1

            for hp in range(4):
                hA, hB = 2 * hp, 2 * hp + 1
                pts = []
                PVA = ps_pv.tile([HD + 1, 512], F32)
                PVB = ps_pv.tile([HD + 1, 512], F32)
                for jt in range(njt):
                    pts.append(emit_qk_jt(ib, hp, jt))
                    if ib == 0 and hp == 0 and jt == 1:
                        blk0[10]()  # v2, v3 before pair0's j-tiles 2..3
                        blk0[11]()
                    if jt % 2 == 1 and jt >= 3:
                        # PVs batched per 2 j-tiles to halve PE tiling-mode
                        # switches (QK runs 64x128 tiles, PV 128x128)
                        for j2 in (jt - 3, jt - 2):
                            pt, _ = pts[j2]
                            emit_pv_jt(ib, hA, j2, pt, 0, PVA, njt)
                            emit_pv_jt(ib, hB, j2, pt, 512, PVB, njt)
                    pace()
                for j2 in (njt - 2, njt - 1):
                    pt, _ = pts[j2]
                    emit_pv_jt(ib, hA, j2, pt, 0, PVA, njt)
                    emit_pv_jt(ib, hB, j2, pt, 512, PVB, njt)
                emit_combine(ib, hA, PVA, nms)
                emit_combine(ib, hB, PVB, nms)
                if ib == 0 and hp < 3:  # q/k groups feeding the next pair
                    blk0[1 + hp]()
                    blk0[5 + hp]()
                if ib == 3:  # ready units to cover each normalize chain
                    for t in range(8 + hp, 12, 4):
                        emit_outproj_tt(t)
                normalize_pair(ib, hA, hB, nms)
                pace()
        for tt in range(12, 16):
            emit_outproj_tt(tt)
    nc.compile()
    return nc


def _make_in_maps(x, w_qkv, w_out):
    bf = ml_dtypes.bfloat16
    # triangular mask for the diagonal 128x128 block: keep i_local >= j_local
    mask = np.where(
        np.arange(P)[None, :] >= np.arange(P)[:, None],
        np.float32(0.0), np.float32(NEG),
    ).astype(np.float32)  # [128, 128]
    in_maps = []
    for c in range(8):
        b, g = c // 2, c % 2
        wq = w_qkv[g * FG:(g + 1) * FG]
        wk = w_qkv[D + g * FG:D + (g + 1) * FG]
        wv = w_qkv[2 * D + g * FG:2 * D + (g + 1) * FG]
        in_maps.append({
            "xT": np.ascontiguousarray(x[b].T).astype(bf),
            "wqkvT": np.ascontiguousarray(
                np.concatenate([wq.T, wk.T, wv.T], axis=1)).astype(bf),
            "woutT": np.ascontiguousarray(w_out[:, g * FG:(g + 1) * FG].T).astype(bf),
            "maskd": mask,
        })
    return in_maps


def _maybe_patch_ldw_opt():
    """Env-gated A/B: rewrite walrus's --enable-ldw-opt=false to =true."""
    import os
    if os.environ.get("ATTN_LDW_OPT") != "1":
        return
    import concourse.bass_utils as bu
    if getattr(bu, "_ldw_patched", False):
        return
    orig = bu.run_command

    def patched(argv, **kw):
        argv = ["--enable-ldw-opt=true" if a == "--enable-ldw-opt=false" else a
                for a in argv]
        return orig(argv, **kw)

    bu.run_command = patched
    bu._ldw_patched = True


def _ensure_ntff_hook():
    """The agent image's antenv package lacks axon_hooks; shim it so
    run_bass_kernel_spmd(trace=True) can capture NTFF profiles."""
    import sys, types
    try:
        import antenv.axon_hooks  # noqa: F401
        return
    except ImportError:
        pass
    import antenv
    mod = types.ModuleType("antenv.axon_hooks")
    mod._hook = None
    def set_axon_ntff_profile_hook(h):
        mod._hook = h
    def get_axon_ntff_profile_hook():
        return mod._hook
    mod.set_axon_ntff_profile_hook = set_axon_ntff_profile_hook
    mod.get_axon_ntff_profile_hook = get_axon_ntff_profile_hook
    sys.modules["antenv.axon_hooks"] = mod
    antenv.axon_hooks = mod
    try:
        from trn_agent_boot.trn_boot import _ntff_profile_via_ctypes
        set_axon_ntff_profile_hook(
            _ntff_profile_via_ctypes("/opt/axon/libaxon_pjrt.so"))
    except Exception as e:  # degrade to no tracing
        print(f"ntff hook install failed: {e}")


def run(x, w_qkv, w_out, trace=False, trace_kwargs=None):
    _import_concourse()
    if trace:
        _ensure_ntff_hook()
    _maybe_patch_ldw_opt()
    from concourse.bass_utils import run_bass_kernel_spmd

    if "nc" not in _CACHE:
        _CACHE["nc"] = _build_nc()
    nc = _CACHE["nc"]
    in_maps = _make_in_maps(np.asarray(x), np.asarray(w_qkv), np.asarray(w_out))
    kw = dict(trace_kwargs or {})
    res = run_bass_kernel_spmd(nc, in_maps, core_ids=list(range(8)), trace=trace, **kw)
    outs = [r["out"] for r in res.results]
    full = np.empty((B, T, D), dtype=np.float32)
    for b in range(B):
        full[b] = outs[2 * b].astype(np.float32) + outs[2 * b + 1].astype(np.float32)
    return full, res


def kernel(x, w_qkv, w_out):
    full, _ = run(x, w_qkv, w_out, trace=False)
    return full
